# revision 23
# baseline (speedup 1.0000x reference)
"""Trainium2 Bass kernel: GQA flash-decoding with paged KV cache (sparse attention).

Problem: B=32 requests, HQ=32 q heads, HKV=8 kv heads, D=128, S=4096 max ctx.
reference = scatter fresh (xk,xv) into kv_buffer at cur_select_index, gather
per-request KV via b_req_tokens_table, masked softmax(q@k^T/sqrt(D)) @ v.

Strategy (request-parallel over 8 cores, no collectives):
 - Host: balance requests across cores by chunk count (4 per core), then
   pack each core's chunk demand into E shared "entries" (row-blocks of
   the 128-wide q/acc space).  A request may span several entries (its
   unnormalized partial acc/l just add up; host sums them), which lets
   the shared per-entry budgets hug each core's actual demand: ~10% less
   HBM traffic than rigid per-slot banding.  KV is shipped bf16 (half
   of f32) pre-transposed:
     KT slab [d=128, (chunk, kv_head, tok)]   -- K with d on partitions
     V  slab [tok=128, (chunk, kv_head, d)]   -- V natural
   The fresh token is shipped in sequence position 0 of each request.
 - Device per 128-token chunk (HWDGE loads grouped into ~2MB DMAs,
   alternating between the sync and scalar rings):
     scores: per kv head, matmul with KT_head as the 128-col stationary
       (FWL, bf16) streaming 4 q columns -> sc[tok, 32 rkg] in PSUM.
     p = exp(sc*scale + mask_bias[token]) via ACT, bf16.
     acc[32 rkg, (head,d)] += p^T @ V  -- p is the 32-col stationary, V
       streams 2x512 cols; cross-head rows are garbage, discarded on host.
     l[rkg] += p^T @ ones.
   Entries alternate between two PSUM acc pairs so entry e+1 accumulates
   while entry e stages out; all staged results leave in ONE final DMA.
 - Host: per request, sum its entries' partials, select head(rkg)'s
   128-col block, divide by l, un-permute.
"""

import os
import sys
from contextlib import ExitStack
from functools import lru_cache

import numpy as np
from ml_dtypes import bfloat16

_REPO = os.environ.get("TRN_RL_REPO", "/opt/trn_rl_repo")
if _REPO not in sys.path:
    sys.path.insert(0, _REPO)

import concourse.bass as bass  # noqa: E402
import concourse.tile as tile  # noqa: E402
from concourse import mybir  # noqa: E402
from concourse.bass_utils import run_bass_kernel_spmd  # noqa: E402

B, HQ, HKV, D, S = 32, 32, 8, 128, 4096
G = HQ // HKV  # 4 q heads per kv head
N_CORES = 8
SLOTS = B // N_CORES  # 4 requests per core
ROW = 2 * HKV * D  # 2048 f32 per kv row (8 K heads + 8 V heads)
HALF = HKV * D  # 1024: one of K / V per row
NEG = np.float32(-1.0e30)
QK_SCALE = float(1.0 / np.sqrt(D))
F32 = mybir.dt.float32
BF16 = mybir.dt.bfloat16
GROUP = 4  # target chunks per DMA (~1MB KT+V per group); groups are
           # balanced within an entry so none is pathologically small.


def _group_sizes(nch, tail=False):
    """Balanced group sizes; for the final entry end on a ~2-chunk group
    so the last compute lags the last DMA arrival as little as possible."""
    if tail and nch > 3:
        return _group_sizes(nch - 2) + [2]
    n_g = max(1, (nch + GROUP - 1) // GROUP)
    base, rem = divmod(nch, n_g)
    return [base + 1] * rem + [base] * (n_g - rem)


def _legalize_waits(nc):
    """This walrus build accepts at most ONE sync wait per instruction
    ("Too many sync wait commands").  Tile's semaphore assignment emits
    multi-waits; hoist all but the last wait of each instruction onto
    freshly inserted same-engine NOPs placed immediately before it (the
    engine blocks at the NOP instead of at the instruction — equivalent)."""
    counter = 0
    for fn in nc.m.functions:
        for bb in fn.blocks:
            out = []
            for inst in bb.instructions:
                si = inst.sync_info
                waits = list(si.on_wait) if (si and si.on_wait) else []
                if len(waits) > 1:
                    for w in waits[:-1]:
                        nop = mybir.InstNoOp(
                            name=f"WSPLIT-{counter}",
                            engine=inst.engine,
                            ins=[],
                            outs=[],
                            sync_info=mybir.SyncInfo(on_wait=[w], on_update=[]),
                        )
                        counter += 1
                        out.append(nop)
                    si.on_wait = [waits[-1]]
                out.append(inst)
            bb.instructions = out
    return counter


def _feasible_assign(shape, budgets):
    """shape: per-request chunk counts (desc).  budgets: entry sizes.
    Return per-request disjoint entry subsets (masks) with subset-sum >=
    demand, or None."""
    E = len(budgets)
    full = (1 << E) - 1

    @lru_cache(maxsize=None)
    def can(i, mask):
        if i == len(shape):
            return ()
        sub = mask
        while sub:
            if sum(budgets[e] for e in range(E) if sub >> e & 1) >= shape[i]:
                rest = can(i + 1, mask & ~sub)
                if rest is not None:
                    return (sub,) + rest
            sub = (sub - 1) & mask
        return None

    return can(0, full)


def _plan(req_len):
    """Balance requests over cores, then find shared entry budgets
    minimizing total shipped chunks such that every core can pack its
    requests (splitting across entries allowed)."""
    ch = ((req_len + 127) // 128).astype(int)
    order = np.argsort(-ch, kind="stable")
    cores = [[] for _ in range(N_CORES)]
    loads = np.zeros(N_CORES, dtype=int)
    for r in order:
        cand = [c for c in range(N_CORES) if len(cores[c]) < SLOTS]
        c = min(cand, key=lambda x: (loads[x], len(cores[x])))
        cores[c].append(int(r))
        loads[c] += int(ch[r])
    shapes = [
        tuple(sorted((int(ch[r]) for r in cores[c]), reverse=True))
        for c in range(N_CORES)
    ]

    def all_feasible(budgets):
        t = tuple(budgets)
        return all(_feasible_assign(s, t) is not None for s in shapes)

    budgets = [max(s[j] for s in shapes) for j in range(SLOTS)]
    while True:
        progress = False
        changed = True
        while changed:
            changed = False
            for e in sorted(range(len(budgets)), key=lambda x: -budgets[x]):
                while budgets[e] > 0:
                    budgets[e] -= 1
                    if all_feasible([b for b in budgets if b > 0]):
                        changed = True
                        progress = True
                    else:
                        budgets[e] += 1
                        break
            budgets = [b for b in budgets if b > 0]
        if len(budgets) < 2 * SLOTS:
            e = int(np.argmax(budgets))
            a = budgets[e] // 2
            if a == 0:
                break
            budgets = budgets[:e] + [budgets[e] - a, a] + budgets[e + 1 :]
            continue
        if not progress:
            break
    budgets = sorted(budgets, reverse=True)

    # per-core packing: entry -> (request, first request-chunk index, count)
    packs = []
    for c in range(N_CORES):
        reqs = sorted(cores[c], key=lambda r: -ch[r])
        masks = _feasible_assign(
            tuple(int(ch[r]) for r in reqs), tuple(budgets)
        )
        assert masks is not None
        pack = [None] * len(budgets)
        for r, mask in zip(reqs, masks):
            m = 0
            need = int(ch[r])
            for e in range(len(budgets)):
                if mask >> e & 1 and m < need:
                    take = min(budgets[e], need - m)
                    pack[e] = (r, m, take)
                    m += take
        packs.append(pack)
    return budgets, packs


def _build_core_inputs(pack, budgets, E, kv_buffer, combined, xq,
                       b_seq_len, b_req_tokens_table, cur_select_index):
    """Build one core's input arrays (pure sharding/marshaling in numpy)."""
    n_ch_total = int(np.sum(budgets))
    slab = np.zeros((n_ch_total * 128, ROW), dtype=np.float32)
    qmat = np.zeros((E * HQ, D), dtype=np.float32)
    maskb = np.full((128, n_ch_total), NEG, dtype=np.float32)

    kv_flat = kv_buffer.reshape(kv_buffer.shape[0], ROW)
    ch0 = np.concatenate([[0], np.cumsum(budgets)]).astype(int)
    for e in range(E):
        if pack[e] is None:
            continue
        req, m0, cnt = pack[e]
        L = int(b_seq_len[req])
        idx = b_req_tokens_table[req, :L]
        sel = int(cur_select_index[req])
        pos = np.nonzero(idx == sel)[0]
        fresh_visible = pos.size > 0
        buf_idx = np.delete(idx, pos) if fresh_visible else idx
        nbuf = buf_idx.shape[0]
        t_valid = 1 + nbuf  # request tokens: 0 = fresh, 1..nbuf = buffer
        qmat[e * HQ : (e + 1) * HQ] = xq[req]
        for lc in range(cnt):
            m = m0 + lc  # request-chunk index: tokens m*128 .. +128
            gc = int(ch0[e]) + lc
            r0 = gc * 128
            t0 = m * 128
            # rows: token t -> fresh (t==0) / buffer row t-1 / zero pad
            lo, hi = max(t0, 1), min(t0 + 128, t_valid)
            if hi > lo:
                bi = buf_idx[lo - 1 : hi - 1]
                dst = slab[r0 + lo - t0 : r0 + hi - t0]
                if bi.size and np.all(np.diff(bi) == 1):
                    dst[:] = kv_flat[bi[0] : bi[0] + bi.size]
                else:
                    dst[:] = kv_flat[bi]
            if t0 == 0:
                slab[r0] = combined[req]
            col = np.where(np.arange(t0, t0 + 128) < t_valid, 0.0, NEG)
            if t0 == 0 and not fresh_visible:
                col[0] = NEG
            maskb[:, gc] = col

    # device layouts, bf16:
    #   kt[d, c*HALF + k*128 + t] = K[chunk c, tok t, head k, d]
    #   v [t, c*HALF + k*128 + d] = V[chunk c, tok t, head k, d]
    kpart = slab[:, :HALF].reshape(n_ch_total, 128, HKV, D)
    vpart = slab[:, HALF:].reshape(n_ch_total, 128, HKV, D)
    kt = np.ascontiguousarray(kpart.transpose(3, 0, 2, 1)).reshape(
        D, n_ch_total * HALF).astype(bfloat16)
    v = np.ascontiguousarray(vpart.transpose(1, 0, 2, 3)).reshape(
        128, n_ch_total * HALF).astype(bfloat16)
    qT = np.ascontiguousarray(qmat.T).astype(bfloat16)
    return {"kt": kt, "v": v, "qT": qT, "maskb": maskb}


def _build_program(budgets):
    """Emit the SPMD Bass program (identical for every core)."""
    E = len(budgets)
    n_ch_total = int(np.sum(budgets))
    ch0 = np.concatenate([[0], np.cumsum(budgets)]).astype(int)
    LCOLS = (E + SLOTS - 1) // SLOTS

    nc = bass.Bass()
    kt_in = nc.declare_dram_parameter("kt", [D, n_ch_total * HALF], BF16, isOutput=False)
    v_in = nc.declare_dram_parameter("v", [128, n_ch_total * HALF], BF16, isOutput=False)
    q_in = nc.declare_dram_parameter("qT", [D, E * HQ], BF16, isOutput=False)
    maskb_in = nc.declare_dram_parameter("maskb", [128, n_ch_total], F32, isOutput=False)
    acc_out = nc.declare_dram_parameter("acc", [HQ, E * HALF], F32, isOutput=True)
    l_out = nc.declare_dram_parameter("l", [128, LCOLS], F32, isOutput=True)

    with tile.TileContext(nc) as tc, ExitStack() as ctx:
        const_pool = ctx.enter_context(tc.tile_pool(name="const", bufs=1))
        kt_pool = ctx.enter_context(tc.tile_pool(name="ktp", bufs=4))
        v_pool = ctx.enter_context(tc.tile_pool(name="vp", bufs=4))
        p_pool = ctx.enter_context(tc.tile_pool(name="p", bufs=4))
        fin_pool = ctx.enter_context(tc.tile_pool(name="fin", bufs=1))

        sc_pool = ctx.enter_context(tc.tile_pool(name="sc", bufs=3, space="PSUM"))
        acc_pool = ctx.enter_context(tc.tile_pool(name="acc", bufs=1, space="PSUM"))
        l_pool = ctx.enter_context(tc.tile_pool(name="l", bufs=1, space="PSUM"))

        ones = const_pool.tile([128, 1], BF16)
        nc.gpsimd.memset(ones[:], 1.0)
        # constants ride the otherwise-idle SWDGE queue so the HWDGE rings
        # start streaming KV immediately.
        maskb = const_pool.tile([128, n_ch_total], F32)
        nc.gpsimd.dma_start(maskb[:], maskb_in[:])
        qT = const_pool.tile([D, E * HQ], BF16)
        nc.gpsimd.dma_start(qT[:], q_in[:])

        # Two PSUM acc pairs (entries alternate) so entry e+1 accumulates
        # while entry e stages out; banks are re-initialized per entry via
        # start=True on the entry's first matmul (each pair owns its banks
        # exclusively).  l is shared across entries at different (row, col)
        # offsets, so it is memset once and accumulated with start=False.
        accs = []
        for pi in range(2):
            a0 = acc_pool.tile([HQ, 512], F32, name=f"acc0_{pi}")
            a1 = acc_pool.tile([HQ, 512], F32, name=f"acc1_{pi}")
            accs.append((a0, a1))
        l_ps = l_pool.tile([128, LCOLS], F32)
        nc.vector.memset(l_ps[:], 0.0)

        stage_pool = ctx.enter_context(tc.tile_pool(name="stg", bufs=2))

        for e in range(E):
            nch = int(budgets[e])
            c0 = int(ch0[e])
            acc0, acc1 = accs[e % 2]
            g0 = 0
            for gsz in _group_sizes(nch, tail=(e == E - 1)):
                col0 = (c0 + g0) * HALF
                ncol = gsz * HALF
                # both slabs on the sync ring: the scalar (ACT) engine runs
                # the per-chunk exps, and an in-order DMA issue there would
                # stall each entry's first load behind the previous entry's
                # exp chain.
                kt_g = kt_pool.tile([D, (GROUP + 1) * HALF], BF16, tag="kt")
                nc.sync.dma_start(kt_g[:, :ncol], kt_in[:, col0 : col0 + ncol])
                v_g = v_pool.tile([128, (GROUP + 1) * HALF], BF16, tag="v")
                nc.sync.dma_start(v_g[:, :ncol], v_in[:, col0 : col0 + ncol])

                for lc in range(gsz):
                    gc = c0 + g0 + lc  # global chunk (maskb column)
                    first = g0 == 0 and lc == 0
                    last = g0 + lc == nch - 1
                    off = lc * HALF

                    # scoresT[tok, (k,g)] per kv head; KT_head is the
                    # 128-col bf16 stationary -> FWL.
                    sc = sc_pool.tile([128, HQ], F32, tag="sc")
                    for k in range(HKV):
                        nc.tensor.matmul(
                            sc[:, k * G : (k + 1) * G],
                            lhsT=kt_g[:, off + k * 128 : off + (k + 1) * 128],
                            rhs=qT[:, e * HQ + k * G : e * HQ + (k + 1) * G],
                            start=True,
                            stop=True,
                        )

                    # p = exp(scoresT * qk_scale + mask_bias[token])
                    p = p_pool.tile([128, HQ], BF16, tag="p")
                    nc.scalar.activation(
                        p[:],
                        sc[:],
                        mybir.ActivationFunctionType.Exp,
                        bias=maskb[:, gc : gc + 1],
                        scale=QK_SCALE,
                    )

                    # acc[rkg, (k,d)] += p^T @ V  (p is the 32-col
                    # stationary; V streams 512 cols per matmul; rows of
                    # acc outside head k's group are garbage -> host
                    # selects the right 128-col block per row).
                    nc.tensor.matmul(
                        acc0[:, :],
                        lhsT=p[:],
                        rhs=v_g[:, off : off + 512],
                        start=first,
                        stop=last,
                        skip_group_check=True,
                    )
                    nc.tensor.matmul(
                        acc1[:, :],
                        lhsT=p[:],
                        rhs=v_g[:, off + 512 : off + HALF],
                        start=first,
                        stop=last,
                        skip_group_check=True,
                    )

                    # l[entry rows] += sum_tok p
                    glob_last = e == E - 1 and last
                    lr = (e % SLOTS) * HQ
                    lcidx = e // SLOTS
                    nc.tensor.matmul(
                        l_ps[lr : lr + HQ, lcidx : lcidx + 1],
                        lhsT=p[:],
                        rhs=ones[:],
                        start=False,
                        stop=glob_last,
                        tile_position=(0, lr),
                        skip_group_check=True,
                    )
                g0 += gsz

            # entry done: stage acc pair out of PSUM (same partitions) and
            # ship it immediately — only the last entry's 128KB is left
            # for the tail.
            stg = stage_pool.tile([HQ, HALF], F32, tag="stg")
            nc.vector.tensor_copy(stg[:, :512], acc0[:])
            nc.vector.tensor_copy(stg[:, 512:], acc1[:])
            nc.sync.dma_start(acc_out[:, e * HALF : (e + 1) * HALF], stg[:])

        l_sb = fin_pool.tile([128, LCOLS], F32)
        nc.vector.tensor_copy(l_sb[:], l_ps[:])
        nc.sync.dma_start(l_out[:], l_sb[:])

    _legalize_waits(nc)
    return nc


def kernel(xq, xk, xv, kv_buffer, cur_select_index, b_req_tokens_table, b_seq_len):
    xq = np.asarray(xq, dtype=np.float32)
    xk = np.asarray(xk, dtype=np.float32)
    xv = np.asarray(xv, dtype=np.float32)
    kv_buffer = np.asarray(kv_buffer, dtype=np.float32)
    cur_select_index = np.asarray(cur_select_index)
    b_req_tokens_table = np.asarray(b_req_tokens_table)
    b_seq_len = np.asarray(b_seq_len)
    assert xq.shape == (B, HQ, D) and kv_buffer.shape == (B * S, 2 * HKV, D)

    # tokens the device processes per request: fresh + buffer rows
    # (buffer rows = seq_len minus the scattered position when visible)
    req_len = np.empty(B, dtype=np.int64)
    for i in range(B):
        L = int(b_seq_len[i])
        idx = b_req_tokens_table[i, :L]
        visible = bool(np.any(idx == int(cur_select_index[i])))
        req_len[i] = L if visible else L + 1

    budgets, packs = _plan(req_len)
    E = len(budgets)
    combined = np.concatenate([xk, xv], axis=1).reshape(B, ROW)

    in_maps = []
    for c in range(N_CORES):
        in_maps.append(
            _build_core_inputs(
                packs[c], budgets, E, kv_buffer, combined, xq,
                b_seq_len, b_req_tokens_table, cur_select_index,
            )
        )

    nc = _build_program(budgets)
    res = run_bass_kernel_spmd(nc, in_maps, core_ids=list(range(N_CORES)))

    # host epilogue: per request, sum its entries' partial acc/l, select
    # head(rkg)'s 128-col block, divide by the softmax denominator.
    blk = (np.arange(HQ) // G)[:, None] * D + np.arange(D)[None, :]  # [32,128]
    out_full = np.zeros((B, HQ, D), dtype=np.float32)
    for c in range(N_CORES):
        acc = np.asarray(res.results[c]["acc"], dtype=np.float32)  # [32, E*1024]
        lv = np.asarray(res.results[c]["l"], dtype=np.float32)     # [128, LCOLS]
        per_req = {}
        for e, slot in enumerate(packs[c]):
            if slot is None:
                continue
            req = slot[0]
            a = acc[:, e * HALF : (e + 1) * HALF]
            li = lv[(e % SLOTS) * HQ : (e % SLOTS + 1) * HQ, e // SLOTS]
            if req in per_req:
                per_req[req][0] += a
                per_req[req][1] += li
            else:
                per_req[req] = [a.copy(), li.copy()]
        for req, (a, li) in per_req.items():
            sel = np.take_along_axis(a, blk, axis=1)  # [32, 128]
            out_full[req] = sel / li[:, None]
    return out_full


if __name__ == "__main__":
    import reference

    ins = {k: np.asarray(v) for k, v in reference.setup_inputs().items()}
    got = kernel(**ins)
    exp = np.asarray(reference.reference(**ins))
    err = np.abs(got - exp).max() / (np.abs(exp).max() + 1e-30)
    print("max abs err:", np.abs(got - exp).max(), "rel:", err)
